# revision 1
# baseline (speedup 1.0000x reference)
"""Causal multi-head attention (B=4, S=2048, D=1024, H=16) on 8 TRN2 cores.

Sharding: data-parallel over batch (4) x tensor-parallel over head groups (2).
Core c handles batch c//2, heads (c%2)*8 .. (c%2)*8+8.  Each core computes a
partial output y_c = attn_out_c @ Wo[rows of its heads]; the host sums the two
partials per batch and adds the bias.

Per-core kernel (matmuls in fp32r: full PE rate at free dim >= 256, inputs
rounded to 1-8-11; psum accumulation in fp32).  Projections and attention are
software-pipelined per 512-wide sequence chunk: chunk n's qT/kT/v projections
are emitted just before chunk n's attention, all pools coexist (no SBUF
reuse barriers), so exp/mask/normalize work overlaps the next chunk's
projection matmuls.

  projections: qT, kT = (x@Wq)^T, (x@Wk)^T in [adim, S] layout; v = x@Wv
           seq-major, augmented with a ones column per head (the ones column
           makes the attn@v matmul also emit the softmax denominators).
  attention (per si-chunk, head): scores^T tiles kT.T @ qT (causal: only
           kj <= diag tiles, diagonal tiles column-restricted), exp on ACT,
           diagonal masking on DVE over live columns, flash-style PSUM
           accumulation of [v|1].T @ exp, normalization via the PE-broadcast
           reciprocal of the sums row; then the output projection rows for
           the chunk (aoT.T @ Wo).
"""

import numpy as np

import concourse.bass as bass
import concourse.mybir as mybir
import concourse.tile as tile
from concourse.bacc import Bacc
from concourse.bass_utils import run_bass_kernel_spmd

F32 = mybir.dt.float32
F32R = mybir.dt.float32r
EXP = mybir.ActivationFunctionType.Exp

B, S, D = 4, 2048, 1024
H, DH = 16, 64
G = 2                # head groups (tensor-parallel factor)
HPC = H // G         # heads per core
AD = HPC * DH        # 512: per-core attention dim
P = 128
NK = D // P          # 8 contraction chunks for the projections
SI = 512             # si (query) chunk width
NCI = S // SI        # 4
NT = S // P          # 16 seq tiles
VW = DH + 1          # 65: v columns + ones column per head
GB = 2               # kj tiles per exp batch (sc psum = GB banks, x2 bufs)


def _emit(nc, tc, xt, wq, wk, wv, wo, masks, y):
    xt_r = xt.rearrange("(k p) (n s) -> n p k s", p=P, s=SI)
    with (
        tc.tile_pool(name="persist", bufs=1) as pp,
        tc.tile_pool(name="qpool", bufs=2) as qpool,
        tc.tile_pool(name="xpool", bufs=2) as xpool,
        tc.tile_pool(name="ao", bufs=2) as aop,
        tc.tile_pool(name="exp", bufs=4) as epool,
        tc.tile_pool(name="small", bufs=2) as spool,
        tc.tile_pool(name="yout", bufs=2) as yp,
        tc.tile_pool(name="ps_u", bufs=2, space="PSUM") as ps_u,
        tc.tile_pool(name="ps_sc", bufs=2, space="PSUM") as ps_sc,
        tc.tile_pool(name="ps_out", bufs=2, space="PSUM") as ps_out,
    ):
        # weight loads split per contraction chunk so the first projection
        # matmuls only wait for their own chunk's DMA, not the full 2MB
        wq_sb = pp.tile([P, NK, AD], F32R)
        wk_sb = pp.tile([P, NK, AD], F32R)
        wv_sb = pp.tile([P, NK, AD], F32R)
        wo_sb = pp.tile([P, AD // P, D], F32R)
        mask_sb = pp.tile([P, 2, SI], F32R)
        wq_r = wq.rearrange("(k p) m -> p k m", p=P)
        wk_r = wk.rearrange("(k p) m -> p k m", p=P)
        wv_r = wv.rearrange("(k p) m -> p k m", p=P)

        def emit_weight_loads():
            # spread the startup weight loads across three DGE issuers so the
            # first projection matmuls aren't serialized behind one queue
            for k in range(NK):
                nc.sync.dma_start(out=wq_sb[:, k, :], in_=wq_r[:, k, :])
            for k in range(NK):
                nc.sync.dma_start(out=wk_sb[:, k, :], in_=wk_r[:, k, :])
            for k in range(NK):
                nc.sync.dma_start(out=wv_sb[:, k, :], in_=wv_r[:, k, :])
            nc.sync.dma_start(out=mask_sb, in_=masks[:, :, :])
            nc.sync.dma_start(out=wo_sb, in_=wo.rearrange("(t p) m -> p t m", p=P))

        kts = [pp.tile([P, AD // P, SI], F32R, name=f"kt{n}") for n in range(NCI)]
        vts = [pp.tile([P, SI // P, HPC, VW], F32R, name=f"vt{n}") for n in range(NCI)]

        def emit_x_load(ci):
            xa = xpool.tile([P, NK // 2, SI], F32R, tag="xt", name="xa")
            for k in range(NK // 2):
                nc.sync.dma_start(out=xa[:, k, :], in_=xt_r[ci][:, k, :])
            xb = xpool.tile([P, NK // 2, SI], F32R, tag="xt", name="xb")
            for k in range(NK // 2):
                nc.sync.dma_start(out=xb[:, k, :], in_=xt_r[ci][:, NK // 2 + k, :])
            return xa, xb

        def emit_proj(ci, xab):
            # ---- projections for chunk n = ci ---------------------------
            xa, xb = xab

            def xk(k):
                return (xa if k < NK // 2 else xb)[:, k % (NK // 2), :]

            qt = qpool.tile([P, AD // P, SI], F32R, name="qt")
            for dst, w_sb, eng in ((qt, wq_sb, nc.vector), (kts[ci], wk_sb, nc.scalar)):
                for m in range(AD // P):
                    ps = ps_u.tile([P, SI], F32, tag="u", name="psp")
                    for k in range(NK):
                        nc.tensor.matmul(
                            ps,
                            w_sb[:, k, m * P : (m + 1) * P],
                            xk(k),
                            start=(k == 0),
                            stop=(k == NK - 1),
                        )
                    if eng is nc.vector:
                        eng.tensor_copy(dst[:, m, :], ps)
                    else:
                        eng.copy(dst[:, m, :], ps)
            for st in range(SI // P):
                ps = ps_u.tile([P, AD], F32, tag="u", name="psv")
                for k in range(NK):
                    nc.tensor.matmul(
                        ps,
                        xk(k)[:, st * P : (st + 1) * P],
                        wv_sb[:, k, :],
                        start=(k == 0),
                        stop=(k == NK - 1),
                    )
                nc.vector.tensor_copy(
                    vts[ci][:, st, :, 0:DH],
                    ps.rearrange("p (h d) -> p h d", d=DH),
                )
                nc.vector.memset(vts[ci][:, st, :, DH : DH + 1].bitcast(F32), 1.0)
            return qt

        def emit_attn(ci, qt, prev):
            # ---- attention for si chunk ci ------------------------------
            aoT = aop.tile([P, AD // P, SI], F32R, name="aoT")
            nkj = 4 * ci + 4
            for h in range(HPC):
                rb = (h % 2) * 64
                tq = h // 2
                outp = ps_out.tile([P, SI], F32, name="outp")
                for g0 in range(0, nkj, GB):
                    gsz = min(GB, nkj - g0)
                    scp = ps_sc.tile([P, GB, SI], F32, name="scp")
                    # diagonal tiles: columns < lo are fully masked; skip them
                    # in scores, exp, mask-mul and attn@v.  live condition
                    # f >= p + 128*jd == (f-lo) >= p + 128*mi.  The whole
                    # exp group uses the group's min lo so the ACT op only
                    # reads psum columns the score matmuls initialized.
                    los = []
                    for j in range(gsz):
                        jd = g0 + j - 4 * ci
                        los.append((min(jd, 2) * P if jd >= 0 else 0, jd))
                    g_lo = min(lo for lo, _ in los)
                    los = [(max(lo, g_lo), jd) for lo, jd in los]
                    for j in range(gsz):
                        kj = g0 + j
                        nc.tensor.matmul(
                            scp[:, j, g_lo:SI],
                            kts[kj // 4][rb : rb + 64, tq, (kj % 4) * P : (kj % 4 + 1) * P],
                            qt[rb : rb + 64, tq, g_lo:SI],
                            start=True,
                            stop=True,
                        )
                    ex = epool.tile([P, GB, SI], F32R, name="ex")
                    nc.scalar.activation(
                        ex[:, 0:gsz, g_lo:SI], scp[:, 0:gsz, g_lo:SI], EXP
                    )
                    for j in range(gsz):
                        kj = g0 + j
                        lo, jd = los[j]
                        if jd >= 0:
                            mi = jd - lo // P  # 0 or 1
                            nc.vector.tensor_mul(
                                ex[:, j, lo:SI],
                                ex[:, j, lo:SI],
                                mask_sb[:, mi, 0 : SI - lo],
                            )
                        nc.tensor.matmul(
                            outp[0:VW, lo:SI],
                            vts[kj // 4][:, kj % 4, h, :],
                            ex[:, j, lo:SI],
                            start=(kj == 0),
                            stop=(kj == nkj - 1),
                        )
                # epilogue: normalize by the sums row (row DH of outp).
                # 1/sums is replicated to 64 partitions via a K=1 PE outer
                # product with an all-ones row (mask1 is all-ones at f>=255).
                rcp = spool.tile([P, SI], F32R, tag="rcp", name="rcp")
                with nc.allow_low_precision("fp32r normalization"):
                    nc.vector.reciprocal(rcp[DH : DH + 1, :], outp[DH : DH + 1, :])
                rep_ps = ps_u.tile([64, SI], F32, tag="u", name="rep_ps")
                nc.tensor.matmul(
                    rep_ps,
                    mask_sb[DH : DH + 1, 1, SI - 64 : SI],
                    rcp[DH : DH + 1, :],
                    start=True,
                    stop=True,
                )
                rep = spool.tile([64, SI], F32, tag="rep", bufs=1, name="rep")
                nc.vector.tensor_copy(rep, rep_ps)
                dst = aoT[rb : rb + 64, tq, :]
                if rb == 0:
                    nc.vector.tensor_mul(dst, outp[0:DH, :], rep)
                else:
                    stg = spool.tile([P, SI], F32R, tag="rcp", name="stg")[0:64, :]
                    nc.vector.tensor_mul(stg, outp[0:DH, :], rep)
                    nc.sync.dma_start(out=dst, in_=stg)
                if prev is not None:
                    emit_p3_unit(ci - 1, prev, h // 2, h % 2)
            return aoT

        def emit_p3_unit(ao_ci, aoT, st, half):
            # one output-projection tile (st, half) for si chunk ao_ci
            ps = ps_u.tile([P, 512], F32, tag="u", name="ps3")
            for t in range(AD // P):
                nc.tensor.matmul(
                    ps,
                    aoT[:, t, st * P : (st + 1) * P],
                    wo_sb[:, t, half * 512 : (half + 1) * 512],
                    start=(t == 0),
                    stop=(t == AD // P - 1),
                )
            ysb = yp.tile([P, 512], F32, name="ysb")
            if half == 0:
                nc.scalar.copy(ysb, ps)
            else:
                nc.vector.tensor_copy(ysb, ps)
            nc.sync.dma_start(
                out=y[
                    ao_ci * SI + st * P : ao_ci * SI + (st + 1) * P,
                    half * 512 : (half + 1) * 512,
                ],
                in_=ysb,
            )

        # first chunk: interleave x and wq chunk loads so the first
        # projection group's dependencies complete earliest
        xa0 = xpool.tile([P, NK // 2, SI], F32R, tag="xt", name="xa")
        xb0 = xpool.tile([P, NK // 2, SI], F32R, tag="xt", name="xb")
        for k in range(NK // 2):
            nc.sync.dma_start(out=xa0[:, k, :], in_=xt_r[0][:, k, :])
            nc.sync.dma_start(out=wq_sb[:, k, :], in_=wq_r[:, k, :])
        for k in range(NK // 2):
            nc.sync.dma_start(out=xb0[:, k, :], in_=xt_r[0][:, NK // 2 + k, :])
            nc.sync.dma_start(out=wq_sb[:, NK // 2 + k, :], in_=wq_r[:, NK // 2 + k, :])
        x0 = (xa0, xb0)
        for k in range(NK):
            nc.sync.dma_start(out=wk_sb[:, k, :], in_=wk_r[:, k, :])
        for k in range(NK):
            nc.sync.dma_start(out=wv_sb[:, k, :], in_=wv_r[:, k, :])
        nc.sync.dma_start(out=mask_sb, in_=masks[:, :, :])
        nc.sync.dma_start(out=wo_sb, in_=wo.rearrange("(t p) m -> p t m", p=P))
        qts = {0: emit_proj(0, x0)}
        prev_ao = None
        for ci in range(NCI):
            if ci + 1 < NCI:
                qts[ci + 1] = emit_proj(ci + 1, emit_x_load(ci + 1))
            prev_ao = emit_attn(ci, qts.pop(ci), prev_ao)
        for st in range(SI // P):
            for half in range(2):
                emit_p3_unit(NCI - 1, prev_ao, st, half)


def build():
    nc = Bacc()
    xt = nc.dram_tensor("xt", [D, S], F32R, kind="ExternalInput")
    wq = nc.dram_tensor("wq", [D, AD], F32R, kind="ExternalInput")
    wk = nc.dram_tensor("wk", [D, AD], F32R, kind="ExternalInput")
    wv = nc.dram_tensor("wv", [D, AD], F32R, kind="ExternalInput")
    wo = nc.dram_tensor("wo", [AD, D], F32R, kind="ExternalInput")
    masks = nc.dram_tensor("masks", [P, 2, SI], F32R, kind="ExternalInput")
    y = nc.dram_tensor("y", [S, D], F32, kind="ExternalOutput")
    with tile.TileContext(nc) as tc:
        _emit(nc, tc, xt, wq, wk, wv, wo, masks, y)
    nc.compile()
    return nc


_NC = None


def _causal_masks():
    p = np.arange(P)[:, None]
    f = np.arange(SI)[None, :]
    return np.stack(
        [(f >= p).astype(np.float32), (f >= p + P).astype(np.float32)], axis=1
    )  # [P, 2, SI]


def run(x, Wq, Wk, Wv, Wo, bo, **run_kwargs):
    global _NC
    x = np.asarray(x, np.float32)
    Wq = np.asarray(Wq, np.float32)
    Wk = np.asarray(Wk, np.float32)
    Wv = np.asarray(Wv, np.float32)
    Wo = np.asarray(Wo, np.float32)
    bo = np.asarray(bo, np.float32)

    if _NC is None:
        _NC = build()

    masks = _causal_masks()
    wq_s = Wq * (1.0 / np.sqrt(DH))  # fold the 1/sqrt(dh) score scale into q
    in_maps = []
    for c in range(2 * B):
        b, g = divmod(c, G)
        cols = slice(g * AD, (g + 1) * AD)
        in_maps.append(
            {
                "xt": np.ascontiguousarray(x[b].T),
                "wq": np.ascontiguousarray(wq_s[:, cols]),
                "wk": np.ascontiguousarray(Wk[:, cols]),
                "wv": np.ascontiguousarray(Wv[:, cols]),
                "wo": np.ascontiguousarray(Wo[cols, :]),
                "masks": masks,
            }
        )

    res = run_bass_kernel_spmd(_NC, in_maps, core_ids=list(range(2 * B)), **run_kwargs)
    ys = [m["y"] for m in res.results]
    out = np.stack([ys[G * b] + ys[G * b + 1] for b in range(B)]) + bo
    return out.astype(np.float32), res


def kernel(**inputs):
    out, _ = run(**inputs)
    return out



# revision 2
# speedup vs baseline: 1.0215x; 1.0215x over previous
"""Causal multi-head attention (B=4, S=2048, D=1024, H=16) on 8 TRN2 cores.

Sharding: data-parallel over batch (4) x tensor-parallel over head groups (2).
Core c handles batch c//2, heads (c%2)*8 .. +8.  Each core computes a partial
output y_c = attn_out_c @ Wo[rows of its heads] in bf16; the host sums the two
partials per batch and adds the bias.

Per-core kernel, fp8/bf16 fast paths (all matmul inputs quantized with
error-compensated hi/lo fp8 splits where precision matters):

  QKV projections: x and Wq/Wk/Wv are split host-side into fp8e4m3 hi+lo
      (w ~ wh+wl, x ~ xh+xl).  Each 128-contraction chunk needs the three
      products wh*xh, wh*xl, wl*xh (the lo*lo term is negligible); pairs of
      products across adjacent k-chunks are packed into fp8 DoubleRow
      matmuls (2 plane-products per instruction at 0.5 cycles/row), so
      K=1024 costs 12 DR instructions vs 8 fp32r ones (0.75x).
  scores: q and k (psum f32) are re-split on-device into fp8 hi/lo.  kT is
      stored partition-duplicated ([Kh;Kh],[Kl;Kl]) and qT stacked
      ([Qh;Ql]) so ONE DoubleRow instruction computes the exact
      (Kh+Kl).T @ (Qh+Ql) per 128-key tile: 2x the fp32r score rate.
  attention: exp on ACT -> bf16; causal masking on DVE (2x mode on bf16);
      attn@v in [q,d] orientation (stationary = exp tile [128k,128q],
      moving = v[128k,65] with a ones column) so the denominators land in
      psum column 64 and normalization is a per-partition tensor_scalar
      multiply -- no PE broadcast matmuls.  65-row moving ops halve the
      fp32r-orientation cost.
  output proj: normalized bf16 attn-out tiles are transposed head-pair-wise
      with batched XBAR DMA transposes ([128q,4t,128d] -> [128d,4t,128q]),
      then aoT.T @ Wo in bf16; y is written bf16 and summed on host in f32.

Software-pipelined per 512-wide chunk as the fp32r baseline: chunk n+1's
projections are emitted before chunk n's attention; output-projection units
for chunk n-1 interleave into chunk n's head loop.  Elementwise work is
spread over DVE and Pool (GPSIMD) so ACT only runs exp.
"""

import ml_dtypes
import numpy as np

import concourse.bass as bass  # noqa: F401
import concourse.mybir as mybir
import concourse.tile as tile
from concourse.bacc import Bacc
from concourse.bass_utils import run_bass_kernel_spmd

F32 = mybir.dt.float32
BF16 = mybir.dt.bfloat16
F8 = mybir.dt.float8e4
EXP = mybir.ActivationFunctionType.Exp
DR = mybir.MatmulPerfMode.DoubleRow

B, S, D = 4, 2048, 1024
H, DH = 16, 64
G = 2                # head groups (tensor-parallel factor)
HPC = H // G         # 8 heads per core
AD = HPC * DH        # 512: per-core attention dim
P = 128
NK = D // P          # 8 contraction chunks for the projections
SI = 512             # si (query) chunk width
NCI = S // SI        # 4
NT = S // P          # 16 seq tiles
VW = DH + 1          # 65: v columns + ones column
GB = 2               # kj tiles per exp batch
NPAIR = HPC // 2     # 4 head pairs (XBAR transpose granularity)
SW = 64.0            # weight pre-scale: lifts U(+-1/32) weights out of fp8
                     # subnormal range; descaled via the exp scale / ones col
SEXP = 1.0 / (SW * SW * 8.0)  # exp reads SW^2-scaled scores; 8 = sqrt(DH)

# (lhsT source, rhs source) term pairs for the 3-term hi/lo projection:
# w*x ~ wh*xh + wh*xl + wl*xh, each accumulated over chunk pairs.
TERMS = ((0, 0), (0, 1), (1, 0))


def _emit(nc, tc, xh, xl, wqh, wql, wkh, wkl, wvh, wvl, wo, masks, y):
    xh_r = xh.rearrange("(k p) (n s) -> n p k s", p=P, s=SI)
    xl_r = xl.rearrange("(k p) (n s) -> n p k s", p=P, s=SI)
    with (
        tc.tile_pool(name="persist", bufs=1) as pp,
        tc.tile_pool(name="xpool", bufs=2) as xpool,
        tc.tile_pool(name="qkst", bufs=2) as qkst,
        tc.tile_pool(name="q8p", bufs=2) as q8p,
        tc.tile_pool(name="exp", bufs=6) as epool,
        tc.tile_pool(name="aos", bufs=3) as aosp,
        tc.tile_pool(name="aot", bufs=4) as aotp,
        tc.tile_pool(name="small", bufs=4) as spool,
        tc.tile_pool(name="yout", bufs=4) as yp,
        tc.tile_pool(name="ps_u", bufs=2, space="PSUM") as ps_u,
        tc.tile_pool(name="ps_sc", bufs=2, space="PSUM") as ps_sc,
        tc.tile_pool(name="ps_av", bufs=2, space="PSUM") as ps_av,
    ):
        w_sb = {}
        for nm in ("qh", "ql", "kh", "kl", "vh", "vl"):
            w_sb[nm] = pp.tile([P, NK, AD], F8, name=f"w{nm}")
        wo_sb = pp.tile([P, AD // P, D], BF16)
        mask_sb = pp.tile([P, 2, SI], BF16)
        kd8 = pp.tile([P, HPC, 2, S], F8, name="kd8")
        vts = pp.tile([P, NT, HPC, VW], BF16, name="vts")

        # SW in the ones column: the denominator then carries the same scale
        # as the numerator, cancelling the v pre-scale.  Emitted once, before
        # any attn@v reads it.
        nc.vector.memset(vts[:, :, :, DH : DH + 1], SW)

        w_dram = {"qh": wqh, "ql": wql, "kh": wkh, "kl": wkl, "vh": wvh, "vl": wvl}
        w_r = {nm: t.rearrange("(k p) m -> p k m", p=P) for nm, t in w_dram.items()}

        def emit_x_load(ci):
            xa = xpool.tile([P, NK, SI], F8, tag="xh", name="xa")
            nc.sync.dma_start(out=xa, in_=xh_r[ci])
            xb = xpool.tile([P, NK, SI], F8, tag="xl", name="xb")
            nc.sync.dma_start(out=xb, in_=xl_r[ci])
            return xa, xb

        def emit_dr_chain(ps, lhs_tiles, rhs_tiles, lslice, rslice):
            """12 DoubleRow matmuls accumulating the 3-term product."""
            n = len(TERMS) * (NK // 2)
            i = 0
            for lsel, rsel in TERMS:
                for c in range(0, NK, 2):
                    nc.tensor.matmul(
                        ps,
                        lhs_tiles[lsel][:, c : c + 2, lslice],
                        rhs_tiles[rsel][:, c : c + 2, rslice],
                        start=(i == 0),
                        stop=(i == n - 1),
                        perf_mode=DR,
                    )
                    i += 1

        def make_proj_units(ci, xab, chunk0=False):
            """Projection for chunk ci as generators of ~300-450ns PE
            micro-bursts, drip-fed between score groups so ACT never
            starves behind a long filler burst.  Order: q m0..3, k m0..3
            (12 DR + hi cast + lo sub each), assembly DMAs, v st0..3.
            chunk0: q/k emitted serially with split assembly (startup)."""
            xa, xb = xab
            qstg = qkst.tile([P, 2, AD // P, SI], F8, tag="qstg", name="qstg")
            kstg = qkst.tile([P, 2, AD // P, SI], F8, tag="kstg", name="kstg")
            q8 = q8p.tile([P, HPC, SI], F8, name="q8")

            def gen_qk_unit(stg, wh_t, wl_t, m):
                ps = ps_u.tile([P, SI], F32, tag="u", name="psp")
                seq = [(ls, rs, c) for ls, rs in TERMS for c in range(0, NK, 2)]
                n = len(seq)
                for i0 in range(0, n, 3):
                    for i, (lsel, rsel, c) in enumerate(seq[i0 : i0 + 3]):
                        nc.tensor.matmul(
                            ps,
                            (wh_t, wl_t)[lsel][:, c : c + 2, m * P : (m + 1) * P],
                            (xa, xb)[rsel][:, c : c + 2, :],
                            start=(i0 + i == 0),
                            stop=(i0 + i == n - 1),
                            perf_mode=DR,
                        )
                    if i0 + 3 < n:
                        yield
                yield
                nc.vector.tensor_copy(stg[:, 0, m, :], ps)
                nc.vector.tensor_sub(stg[:, 1, m, :], ps, stg[:, 0, m, :])

            def gen_v_unit(st):
                ps = ps_u.tile([P, AD], F32, tag="u", name="psv")
                seq = [(ls, rs, c) for ls, rs in TERMS for c in range(0, NK, 2)]
                n = len(seq)
                for i0 in range(0, n, 3):
                    for i, (lsel, rsel, c) in enumerate(seq[i0 : i0 + 3]):
                        nc.tensor.matmul(
                            ps,
                            (xa, xb)[lsel][:, c : c + 2, st * P : (st + 1) * P],
                            (w_sb["vh"], w_sb["vl"])[rsel][:, c : c + 2, :],
                            start=(i0 + i == 0),
                            stop=(i0 + i == n - 1),
                            perf_mode=DR,
                        )
                    if i0 + 3 < n:
                        yield
                yield
                nc.vector.tensor_copy(
                    vts[:, 4 * ci + st, :, 0:DH],
                    ps.rearrange("p (h d) -> p h d", d=DH),
                )

            def emit_asm(ms):
                # q8 single plane [Qh;Ql] per head (scores broadcast it); kd8
                # planes ([Kh;Kh],[Kl;Kl]): the (hi,lo) staging pair IS the
                # per-plane source, partition-duplicated by two DMAs
                cols = slice(ci * SI, (ci + 1) * SI)
                hs = slice(2 * ms.start, 2 * ms.stop, 2)
                ho = slice(2 * ms.start + 1, 2 * ms.stop, 2)
                nc.sync.dma_start(out=q8[0:64, hs, :], in_=qstg[0:64, 0, ms, :])
                nc.sync.dma_start(out=q8[64:128, hs, :], in_=qstg[0:64, 1, ms, :])
                nc.sync.dma_start(out=q8[0:64, ho, :], in_=qstg[64:128, 0, ms, :])
                nc.sync.dma_start(out=q8[64:128, ho, :], in_=qstg[64:128, 1, ms, :])
                for pl in range(2):  # plane 0 <- hi, plane 1 <- lo
                    for par, src_rows in ((0, slice(0, 64)), (1, slice(64, 128))):
                        hp = hs if par == 0 else ho
                        for dst_rows in (slice(0, 64), slice(64, 128)):
                            nc.sync.dma_start(
                                out=kd8[dst_rows, hp, pl, cols],
                                in_=kstg[src_rows, pl, ms, :],
                            )

            def gen_asm(ms):
                emit_asm(ms)
                yield

            gens = []
            if chunk0:
                # startup: serial q/k with halved assembly so head-0 scores
                # can begin while the second half projects; v units drip into
                # attn(0) (attn@v tracks per-tile v deps)
                nm_half = max(1, AD // P // 2)
                for m0 in range(0, AD // P, nm_half):
                    ms = slice(m0, min(m0 + nm_half, AD // P))
                    for m in range(ms.start, ms.stop):
                        for _ in gen_qk_unit(qstg, w_sb["qh"], w_sb["ql"], m):
                            pass
                    for m in range(ms.start, ms.stop):
                        for _ in gen_qk_unit(kstg, w_sb["kh"], w_sb["kl"], m):
                            pass
                    emit_asm(ms)
                # chunk 0 consumes its own v tiles almost immediately --
                # run the v units serially instead of drip-feeding them
                for st in range(SI // P):
                    for _ in gen_v_unit(st):
                        pass
                return gens, q8
            else:
                for m in range(AD // P):
                    gens.append(gen_qk_unit(qstg, w_sb["qh"], w_sb["ql"], m))
                for m in range(AD // P):
                    gens.append(gen_qk_unit(kstg, w_sb["kh"], w_sb["kl"], m))
                gens.append(gen_asm(slice(0, AD // P)))
            for st in range(SI // P):
                gens.append(gen_v_unit(st))
            return gens, q8

        def emit_norm(h, avp, aos, on_act=False):
            # batched reciprocal of the 4 denominator columns, then one
            # per-partition-scalar multiply per q-tile (on ACT during the
            # drain, when exp work has run out)
            hc = (h % 2) * DH
            rcp = spool.tile([P, SI // P], F32, tag="rcp", name="rcp")
            nc.vector.reciprocal(rcp, avp[:, :, DH : DH + 1])
            for t in range(SI // P):
                if on_act:
                    nc.scalar.activation(
                        aos[:, t, hc : hc + DH],
                        avp[:, t, 0:DH],
                        mybir.ActivationFunctionType.Copy,
                        scale=rcp[:, t : t + 1],
                    )
                else:
                    nc.vector.tensor_scalar_mul(
                        aos[:, t, hc : hc + DH], avp[:, t, 0:DH], rcp[:, t : t + 1]
                    )

        def emit_xbar_pair(aoT, aos, pr):
            nc.sync.dma_start(
                out=aoT[:, pr, :, :],
                in_=aos.rearrange("p t c -> p (t c)"),
                transpose=True,
            )

        def emit_xbars(prev_aos):
            aoT = aotp.tile([P, NPAIR, SI // P, P], BF16, name="aoT")
            for pr in range(NPAIR):
                emit_xbar_pair(aoT, prev_aos[pr], pr)
            return aoT

        def emit_attn(ci, q8, nxt_units, p3_jobs):
            last = ci == NCI - 1
            if last:
                aoT_own = aotp.tile([P, NPAIR, SI // P, P], BF16, name="aoTl")
            aos_list = []
            norm_q = []
            # interleave schedule: later chunks are exp(ACT)-bound, so PE
            # filler work (next proj units + deferred output-proj jobs) is
            # spread across the head loop
            filler = list(nxt_units) + list(p3_jobs)
            nu = len(filler)
            nkj = 4 * ci + 4
            for h in range(HPC):
                for u in filler[h * nu // HPC : (h + 1) * nu // HPC]:
                    u()
                if h % 2 == 0:
                    aos = aosp.tile([P, SI // P, P], BF16, name="aos")
                    aos_list.append(aos)
                avp = ps_av.tile([P, SI // P, VW], F32, name="avp")
                for g0 in range(0, nkj, GB):
                    gsz = min(GB, nkj - g0)
                    los = []
                    for j in range(gsz):
                        jd = g0 + j - 4 * ci
                        los.append((min(jd, 2) * P if jd >= 0 else 0, jd))
                    g_lo = min(lo for lo, _ in los)
                    los = [(max(lo, g_lo), jd) for lo, jd in los]
                    scp = ps_sc.tile([P, GB, SI], F32, name="scp")
                    for j in range(gsz):
                        kj = g0 + j
                        rhs = (
                            q8[:, h, g_lo:SI]
                            .rearrange("p (o s) -> p o s", o=1)
                            .to_broadcast((P, 2, SI - g_lo))
                        )
                        nc.tensor.matmul(
                            scp[:, j, g_lo:SI],
                            kd8[:, h, :, kj * P : (kj + 1) * P],
                            rhs,
                            start=True,
                            stop=True,
                            perf_mode=DR,
                        )
                    ex = epool.tile([P, GB, SI], BF16, name="ex")
                    nc.scalar.activation(
                        ex[:, 0:gsz, g_lo:SI], scp[:, 0:gsz, g_lo:SI], EXP,
                        scale=SEXP,
                    )
                    for j in range(gsz):
                        kj = g0 + j
                        lo, jd = los[j]
                        if jd >= 0:
                            mi = jd - lo // P  # 0 or 1
                            meng = nc.gpsimd if h % 2 == 0 else nc.vector
                            meng.tensor_mul(
                                ex[:, j, lo:SI],
                                ex[:, j, lo:SI],
                                mask_sb[:, mi, 0 : SI - lo],
                            )
                        # one psum accumulation group for the whole bank: the
                        # first matmul's start zeroes the 2KB region, the very
                        # last one stops it (stop is sim-only bookkeeping)
                        for t in range(max(jd, 0), SI // P):
                            nc.tensor.matmul(
                                avp[:, t, :],
                                ex[:, j, t * P : (t + 1) * P],
                                vts[:, kj, h, :],
                                start=(kj == 0 and t == 0),
                                stop=(kj == nkj - 1 and t == SI // P - 1),
                                skip_group_check=True,
                            )
                # defer this head's normalize until the next head's attention
                # is emitted, so the DVE queue never parks on head-end psum
                norm_q.append((h, avp, aos))
                if len(norm_q) > 1:
                    hd = norm_q.pop(0)
                    emit_norm(*hd)
                    if last and hd[0] % 2 == 1:
                        # last chunk: transpose each pair as soon as both its
                        # heads are normalized so the drain overlaps
                        emit_xbar_pair(aoT_own, hd[2], hd[0] // 2)
            while norm_q:
                hd = norm_q.pop(0)
                emit_norm(*hd, on_act=last)
                if last and hd[0] % 2 == 1:
                    emit_xbar_pair(aoT_own, hd[2], hd[0] // 2)
            if last:
                for st in range(SI // P):
                    emit_p3_unit(ci, aoT_own, st)
            return aos_list

        def gen_p3_unit(ao_ci, aoT, st, on_act=False):
            ysb = yp.tile([P, D], BF16, name="ysb")
            for half in range(2):
                ps = ps_u.tile([P, 512], F32, tag="u", name="ps3")
                for pr0 in range(0, NPAIR, 2):
                    for pr in range(pr0, min(pr0 + 2, NPAIR)):
                        nc.tensor.matmul(
                            ps,
                            aoT[:, pr, st, :],
                            wo_sb[:, pr, half * 512 : (half + 1) * 512],
                            start=(pr == 0),
                            stop=(pr == NPAIR - 1),
                        )
                    yield
                if on_act:
                    nc.scalar.copy(ysb[:, half * 512 : (half + 1) * 512], ps)
                else:
                    nc.vector.tensor_copy(ysb[:, half * 512 : (half + 1) * 512], ps)
            nc.sync.dma_start(
                out=y[ao_ci * SI + st * P : ao_ci * SI + (st + 1) * P, :],
                in_=ysb,
            )

        # startup: interleave the first x chunk with wq so the first
        # projection group's dependencies complete earliest
        xa0 = xpool.tile([P, NK, SI], F8, tag="xh", name="xa")
        xb0 = xpool.tile([P, NK, SI], F8, tag="xl", name="xb")
        nc.sync.dma_start(out=w_sb["qh"], in_=w_r["qh"])
        nc.sync.dma_start(out=xa0, in_=xh_r[0])
        nc.sync.dma_start(out=xb0, in_=xl_r[0])
        nc.sync.dma_start(out=w_sb["ql"], in_=w_r["ql"])
        for nm in ("kh", "kl", "vh", "vl"):
            nc.sync.dma_start(out=w_sb[nm], in_=w_r[nm])
        nc.sync.dma_start(out=mask_sb, in_=masks[:, :, :])
        nc.sync.dma_start(out=wo_sb, in_=wo.rearrange("(t p) m -> p t m", p=P))

        units0, q8_0 = make_proj_units(0, (xa0, xb0))
        for u in units0:
            u()
        q8s = {0: q8_0}
        prev_aos = None
        pending = []  # deferred output-projection job thunks

        def make_p3_jobs(ao_ci, aoT):
            return [
                (lambda st=st: emit_p3_unit(ao_ci, aoT, st))
                for st in range(SI // P)
            ]

        for ci in range(NCI):
            nxt_units = []
            if ci + 1 < NCI:
                nxt_units, q8s[ci + 1] = make_proj_units(ci + 1, emit_x_load(ci + 1))
            if prev_aos is not None:
                aoT_prev = emit_xbars(prev_aos)
                pending.extend(make_p3_jobs(ci - 1, aoT_prev))
            # run deferred p3 jobs in the exp-bound later windows: chunk 2
            # hosts chunk 0's, the last chunk hosts the rest
            if ci == 2:
                jobs, pending = pending[: SI // P], pending[SI // P :]
            elif ci == NCI - 1:
                jobs, pending = pending, []
            else:
                jobs = []
            prev_aos = emit_attn(ci, q8s.pop(ci), nxt_units, jobs)
